# revision 1
# baseline (speedup 1.0000x reference)
"""Trainium2 Bass kernel for nn_ConvAttention.

The reference computes:
    fx = conv1x1(x, wf) + bf          # [B,1,H,W]
    gx = conv1x1(x, wg) + bg
    hx = conv1x1(x, wh) + bh
    a  = softmax(fx @ gx, axis=1)     # axis of size 1 -> identically 1.0
    o  = (hx @ a) * x                 # hx @ ones = row-sum broadcast over W

Because the softmax is over a size-1 axis it is exactly 1.0 everywhere, so
    o[b,c,i,j] = s[b,i] * x[b,c,i,j]
    s[b,i]     = sum_c sum_k x[b,c,i,k] * wh[c] + W * bh
wf/bf/wg/bg do not affect the output. The kernel streams x once through
SBUF (read 16 MiB + write 16 MiB per core) - purely memory bound.

Sharding: pure data parallel over batch; 4 batches per core on 8 cores.
Weights (wh, bh) replicated.

Per-core layout: for each (batch, c-chunk of 128, h-half of 32) an SBUF
tile [128 part = channels, 2048 free = 32*64 h,w] - contiguous 8 KiB per
partition in DRAM, 1 MiB per DMA -> max DMA efficiency. The channel
contraction hx = wh^T @ x runs on the TensorEngine (PSUM [1, h*w]
accumulated over the two c-chunks), the w row-sum of hx on VectorE, the
broadcast of s back to 128 partitions via a K=1 matmul with a ones
vector, then an in-place broadcast multiply (VectorE) and store.
"""

from contextlib import ExitStack

import numpy as np

B, C, H, W = 32, 256, 64, 64
N_CORES = 8
BS = B // N_CORES  # batches per core

_CACHE = {}


def _split_multi_waits(nc, mybir):
    """Walrus codegen allows only one sync-wait slot on most instruction
    encodings ("Too many sync wait commands"). Tile's sem assigner sometimes
    attaches 2-3. Hoist the extras onto standalone EventSemaphore
    instructions immediately before, on the same engine - semantically
    identical since engines execute their stream in order."""
    n = 0
    for f in nc.m.functions:
        for bb in f.blocks:
            new_insts = []
            for inst in bb.instructions:
                si = inst.sync_info
                ow = list(si.on_wait) if si and si.on_wait else []
                if len(ow) > 1:
                    for wv in ow[:-1]:
                        n += 1
                        evs = mybir.InstEventSemaphore(
                            name=f"evs_split_{n}",
                            ins=[],
                            outs=[],
                            engine=inst.engine,
                            bass_nofuse=True,
                            sync_info=mybir.SyncInfo(on_wait=[wv], on_update=[]),
                        )
                        nc.register_instruction(evs, overwrite=True)
                        new_insts.append(evs)
                    inst.sync_info = mybir.SyncInfo(
                        on_wait=[ow[-1]],
                        on_update=list(si.on_update) if si.on_update else [],
                    )
                new_insts.append(inst)
            bb.instructions = new_insts
    return n


def _build(bs, c, h, w):
    import concourse.bass as bass
    import concourse.tile as tile
    from concourse import mybir

    f32 = mybir.dt.float32
    P = 128
    n_ch = c // P
    assert c % P == 0
    hw = h * w
    # h-half tile: [P, hh*w], one DMA each; 1 MiB at full size
    n_half = 2 if h % 2 == 0 and (h // 2) * w % 512 == 0 else 1
    hh = h // n_half
    fh = hh * w  # free elems per half-tile
    # PSUM contraction quarters: [1, qf] regions reduced on DVE
    MMN = 512  # max matmul free dim
    qf = min(2 * MMN, fh)  # elems per psum tile (<= 2 banks)
    n_q = hw // qf
    hq = qf // w  # h rows per psum quarter
    mmn = min(MMN, qf)  # free dim per matmul

    nc = bass.Bass("TRN2", target_bir_lowering=False, debug=False)
    x = nc.dram_tensor("x", [bs, c, h, w], f32, kind="ExternalInput").ap()
    wh = nc.dram_tensor("wh", [c], f32, kind="ExternalInput").ap()
    bh = nc.dram_tensor("bh", [1], f32, kind="ExternalInput").ap()
    o = nc.dram_tensor("o", [bs, c, h, w], f32, kind="ExternalOutput").ap()

    X = mybir.AxisListType.X

    with tile.TileContext(nc) as tc, ExitStack() as ctx:
        consts = ctx.enter_context(tc.tile_pool(name="consts", bufs=1))
        xpool = ctx.enter_context(
            tc.tile_pool(name="xp", bufs=bs * n_ch * n_half)
        )
        sp = ctx.enter_context(tc.tile_pool(name="s", bufs=4))
        bcp = ctx.enter_context(tc.tile_pool(name="bc", bufs=4))
        qpp = ctx.enter_context(tc.tile_pool(name="qp", bufs=3, space="PSUM"))
        pbp = ctx.enter_context(tc.tile_pool(name="pb", bufs=2, space="PSUM"))

        # wh as [128, n_ch]: column j holds wh[j*128:(j+1)*128].
        # Bounce through a DVE copy so the first matmul's producers sit on
        # fewer distinct semaphores.
        wh_raw = consts.tile([P, n_ch], f32)
        nc.sync.dma_start(wh_raw[:], wh.rearrange("(j p) -> p j", p=P))
        wh_sb = consts.tile([P, n_ch], f32)
        nc.vector.tensor_copy(wh_sb[:], wh_raw[:])
        # W*bh replicated on all partitions, for the final bias add
        bh_sb = consts.tile([P, 1], f32)
        nc.sync.dma_start(bh_sb[:], bh.to_broadcast((P, 1)))
        biasW = consts.tile([P, 1], f32)
        nc.scalar.mul(biasW[:], bh_sb[:], float(w))
        ones_sb = consts.tile([1, P], f32)
        nc.vector.memset(ones_sb[:], 1.0)

        # Each (batch, h-half) group is a fully independent pipeline:
        # 2 loads (one per c-chunk) -> PE contraction -> w row-sums ->
        # broadcast -> 2 in-place multiplies -> 2 stores. Fine granularity
        # lets the store stream start ~2 MiB after the first load.
        n_qg = fh // qf  # psum tiles per group
        for b in range(bs):
            for hf in range(n_half):
                xts = []
                for ch in range(n_ch):
                    xt = xpool.tile([P, fh], f32)
                    nc.sync.dma_start(
                        xt[:],
                        x[
                            b, ch * P : (ch + 1) * P, hf * hh : (hf + 1) * hh
                        ].rearrange("c h w -> c (h w)"),
                    )
                    xts.append(xt)

                # hx[f] = sum_c wh[c]*x[c,f] on PE, PSUM [1, qf] regions
                # accumulated over c-chunks; then w row-sums on DVE -> s_g
                s_g = sp.tile([1, hh], f32)
                for q in range(n_qg):
                    psq = qpp.tile([1, qf], f32)
                    for n in range(qf // mmn):
                        f0 = q * qf + n * mmn  # offset within the group
                        for ch in range(n_ch):
                            nc.tensor.matmul(
                                psq[:, n * mmn : (n + 1) * mmn],
                                lhsT=wh_sb[:, ch : ch + 1],
                                rhs=xts[ch][:, f0 : f0 + mmn],
                                start=(ch == 0),
                                stop=(ch == n_ch - 1),
                            )
                    nc.vector.reduce_sum(
                        s_g[:, q * hq : (q + 1) * hq],
                        psq[:].rearrange("p (h w) -> p h w", w=w),
                        axis=X,
                    )

                # broadcast s to all 128 partitions via K=1 matmul with
                # ones, add W*bh during the PSUM->SBUF copy
                psum_b = pbp.tile([P, hh], f32)
                nc.tensor.matmul(
                    psum_b[:],
                    lhsT=ones_sb[:1, :],
                    rhs=s_g[:1, :],
                    start=True,
                    stop=True,
                )
                s128 = bcp.tile([P, hh], f32)
                nc.scalar.add(s128[:], psum_b[:], biasW[:])

                # o = s * x in place, then store
                for ch in range(n_ch):
                    xv = xts[ch][:].rearrange("c (h w) -> c h w", w=w)
                    nc.vector.tensor_mul(
                        xv, xv, s128[:, :, None].broadcast_to((P, hh, w))
                    )
                    nc.scalar.dma_start(
                        o[
                            b, ch * P : (ch + 1) * P, hf * hh : (hf + 1) * hh
                        ].rearrange("c h w -> c (h w)"),
                        xts[ch][:],
                    )
    _split_multi_waits(nc, mybir)
    return nc


def get_nc(bs=BS, c=C, h=H, w=W):
    key = (bs, c, h, w)
    if key not in _CACHE:
        _CACHE[key] = _build(bs, c, h, w)
    return _CACHE[key]


def kernel(x, wf, bf, wg, bg, wh, bh, **_unused):
    from concourse.bass_utils import run_bass_kernel_spmd

    x = np.ascontiguousarray(np.asarray(x, dtype=np.float32))
    wh = np.ascontiguousarray(np.asarray(wh, dtype=np.float32))
    bh = np.ascontiguousarray(np.asarray(bh, dtype=np.float32))

    in_maps = [
        {"x": x[k * BS : (k + 1) * BS], "wh": wh, "bh": bh} for k in range(N_CORES)
    ]
    # Tile scheduling is nondeterministic build-to-build and a rare schedule
    # can deadlock on hardware (NRT unrecoverable). Rebuilding produces a
    # fresh schedule, so retry with a clean build on any execution failure.
    last_err = None
    for attempt in range(3):
        try:
            nc = get_nc()
            res = run_bass_kernel_spmd(nc, in_maps, core_ids=list(range(N_CORES)))
            return np.concatenate(
                [res.results[k]["o"] for k in range(N_CORES)], axis=0
            )
        except Exception as e:  # rebuild with a new schedule and retry
            last_err = e
            _CACHE.clear()
    raise last_err



# revision 4
# speedup vs baseline: 1.0328x; 1.0328x over previous
"""Trainium2 Bass kernel for nn_ConvAttention.

The reference computes:
    fx = conv1x1(x, wf) + bf          # [B,1,H,W]
    gx = conv1x1(x, wg) + bg
    hx = conv1x1(x, wh) + bh
    a  = softmax(fx @ gx, axis=1)     # axis of size 1 -> identically 1.0
    o  = (hx @ a) * x                 # hx @ ones = row-sum broadcast over W

Because the softmax is over a size-1 axis it is exactly 1.0 everywhere, so
    o[b,c,i,j] = s[b,i] * x[b,c,i,j]
    s[b,i]     = sum_c sum_k x[b,c,i,k] * wh[c] + W * bh
wf/bf/wg/bg do not affect the output. The kernel streams x once through
SBUF (read 16 MiB + write 16 MiB per core) - purely memory bound. The
fabric (SBUF AXI, ~435 GB/s/core) is the roofline; the goal is to keep the
two HWDGE queues (loads on Sync, stores on Scalar) saturated end to end.

Sharding: pure data parallel over batch; 4 batches per core on 8 cores.
Weights (wh, bh) replicated.

Per-core layout: for each (batch, c-chunk of 128, h-half of 32) an SBUF
tile [128 part = channels, 2048 free = 32*64 h,w] - contiguous 8 KiB per
partition in DRAM, 1 MiB per DMA.

Compute per (batch, h-half) group - kept OFF the DMA critical path:
  1. DVE tensor_reduce over w: [128, hh, 64] -> y[128, hh]   (all lanes)
  2. PE: 3 tiny matmuls into PSUM pb[128, hh]:
       bias:  lhsT=biasB[128,128] (W*bh/128 everywhere), rhs=ones[128,hh]
       chunk0/1: lhsT=whB[:,ch] ([128,128], every column = wh chunk),
                 rhs=y_ch  -> accumulates s[h] replicated on all 128 parts
     One matmul chain does contraction + partition-broadcast + bias.
  3. muls: ch0 on DVE reading pb straight from PSUM; ch1 on GpSimd
     reading an SBUF copy (GpSimd cannot touch PSUM).
  4. stores issued from the Scalar engine (its only job, so a store
     waiting on a mul never blocks unrelated work).
"""

from contextlib import ExitStack

import numpy as np

B, C, H, W = 32, 256, 64, 64
N_CORES = 8
BS = B // N_CORES  # batches per core

_CACHE = {}


def _split_multi_waits(nc, mybir):
    """Walrus codegen allows only one sync-wait slot on most instruction
    encodings ("Too many sync wait commands"). Tile's sem assigner sometimes
    attaches 2-3. Hoist the extras onto standalone EventSemaphore
    instructions immediately before, on the same engine - semantically
    identical since engines execute their stream in order."""
    n = 0
    for f in nc.m.functions:
        for bb in f.blocks:
            new_insts = []
            for inst in bb.instructions:
                si = inst.sync_info
                ow = list(si.on_wait) if si and si.on_wait else []
                if len(ow) > 1:
                    for wv in ow[:-1]:
                        n += 1
                        evs = mybir.InstEventSemaphore(
                            name=f"evs_split_{n}",
                            ins=[],
                            outs=[],
                            engine=inst.engine,
                            bass_nofuse=True,
                            sync_info=mybir.SyncInfo(on_wait=[wv], on_update=[]),
                        )
                        nc.register_instruction(evs, overwrite=True)
                        new_insts.append(evs)
                    inst.sync_info = mybir.SyncInfo(
                        on_wait=[ow[-1]],
                        on_update=list(si.on_update) if si.on_update else [],
                    )
                new_insts.append(inst)
            bb.instructions = new_insts
    return n


def _build(bs, c, h, w):
    import concourse.bass as bass
    import concourse.tile as tile
    from concourse import mybir

    f32 = mybir.dt.float32
    P = 128
    n_ch = c // P
    assert c % P == 0
    n_half = 2 if h % 2 == 0 else 1
    hh = h // n_half
    fh = hh * w  # free elems per tile

    nc = bass.Bass("TRN2", target_bir_lowering=False, debug=False)
    x = nc.dram_tensor("x", [bs, c, h, w], f32, kind="ExternalInput").ap()
    wh = nc.dram_tensor("wh", [c], f32, kind="ExternalInput").ap()
    bh = nc.dram_tensor("bh", [1], f32, kind="ExternalInput").ap()
    o = nc.dram_tensor("o", [bs, c, h, w], f32, kind="ExternalOutput").ap()

    X = mybir.AxisListType.X

    with tile.TileContext(nc) as tc, ExitStack() as ctx:
        consts = ctx.enter_context(tc.tile_pool(name="consts", bufs=1))
        xpool = ctx.enter_context(
            tc.tile_pool(name="xp", bufs=bs * n_ch * n_half)
        )
        ypool = ctx.enter_context(tc.tile_pool(name="yp", bufs=6))
        spool = ctx.enter_context(tc.tile_pool(name="sp", bufs=4))
        pbp = ctx.enter_context(tc.tile_pool(name="pb", bufs=4, space="PSUM"))

        # ---- x loads own the Sync HWDGE queue exclusively; the tiny const
        # DMAs ride the Scalar queue, which is idle until stores begin ----
        tiles = []  # (b, hf) -> [tile per ch]
        order = [(b, hf) for b in range(bs) for hf in range(n_half)]

        def load_tile(b, hf, ch):
            xt = xpool.tile([P, fh], f32)
            nc.sync.dma_start(
                xt[:],
                x[
                    b, ch * P : (ch + 1) * P, hf * hh : (hf + 1) * hh
                ].rearrange("c h w -> c (h w)"),
            )
            return xt

        first = load_tile(*order[0], 0)

        # ---- constants (DMAs on Scalar queue; build ops on GpSimd so the
        # Vector engine's stream starts with real reduction work) ----
        # wh as [128, n_ch]: column j holds wh[j*128:(j+1)*128]
        wh_sb = consts.tile([P, n_ch], f32)
        nc.scalar.dma_start(wh_sb[:], wh.rearrange("(j p) -> p j", p=P))
        bh_sb = consts.tile([P, 1], f32)
        nc.scalar.dma_start(bh_sb[:], bh.to_broadcast((P, 1)))
        # whB[:, ch*128+m] = wh[ch*128+p] for every m: one matmul both
        # contracts over partitions and replicates the result on all 128
        biasW = consts.tile([P, 1], f32)
        nc.gpsimd.tensor_scalar_mul(biasW[:], bh_sb[:], float(w) / P)
        whB = consts.tile([P, n_ch * P], f32)
        for ch in range(n_ch):
            nc.gpsimd.tensor_copy(
                whB[:, ch * P : (ch + 1) * P],
                wh_sb[:, ch : ch + 1].broadcast_to((P, P)),
            )
        biasB = consts.tile([P, P], f32)
        nc.gpsimd.tensor_copy(biasB[:], biasW[:].broadcast_to((P, P)))
        ones_sb = consts.tile([P, hh], f32)
        nc.gpsimd.memset(ones_sb[:], 1.0)

        # ---- rest of the load stream ----
        for i, (b, hf) in enumerate(order):
            row = []
            for ch in range(n_ch):
                if i == 0 and ch == 0:
                    row.append(first)
                else:
                    row.append(load_tile(b, hf, ch))
            tiles.append(row)

        # ---- per-group pipeline ----
        for i, (b, hf) in enumerate(order):
            xts = tiles[i]
            # 1) w row-sums on DVE, all 128 lanes busy
            ys = []
            for ch in range(n_ch):
                y = ypool.tile([P, hh], f32)
                nc.vector.reduce_sum(
                    y[:], xts[ch][:].rearrange("c (h w) -> c h w", w=w), axis=X
                )
                ys.append(y)
            # 2) contraction + broadcast + bias in one PSUM accumulation
            pb = pbp.tile([P, hh], f32)
            nc.tensor.matmul(
                pb[:], lhsT=biasB[:], rhs=ones_sb[:], start=True, stop=False
            )
            for ch in range(n_ch):
                nc.tensor.matmul(
                    pb[:],
                    lhsT=whB[:, ch * P : (ch + 1) * P],
                    rhs=ys[ch][:],
                    start=False,
                    stop=(ch == n_ch - 1),
                )
            # 3) o = s * x in place, split between DVE (reads s straight
            # from PSUM) and GpSimd (reads the SBUF copy; no PSUM access).
            # DVE also carries the reduces, so it takes only ~0.4 tiles of
            # multiply work per group; GpSimd takes the rest - both engines
            # land at ~5.7us/group, under the ~9.9us steady load cadence.
            # For the final group split 50/50 to shorten the tail.
            s128 = spool.tile([P, hh], f32)
            nc.vector.tensor_copy(s128[:], pb[:])
            last = i == len(order) - 1
            # dve_cols[ch]: leading h-rows of tile ch multiplied on DVE
            if last:
                dve_rows = [hh // 2, hh // 2]
            else:
                dve_rows = [(hh * 7) // 16, 0]
            for ch in range(n_ch):
                xv = xts[ch][:].rearrange("c (h w) -> c h w", w=w)
                d = dve_rows[ch] if ch < len(dve_rows) else 0
                if d > 0:
                    nc.vector.tensor_mul(
                        xv[:, :d],
                        xv[:, :d],
                        pb[:, :d, None].broadcast_to((P, d, w)),
                    )
                if d < hh:
                    nc.gpsimd.tensor_mul(
                        xv[:, d:],
                        xv[:, d:],
                        s128[:, d:, None].broadcast_to((P, hh - d, w)),
                    )
                nc.scalar.dma_start(
                    o[
                        b, ch * P : (ch + 1) * P, hf * hh : (hf + 1) * hh
                    ].rearrange("c h w -> c (h w)"),
                    xts[ch][:],
                )
    _split_multi_waits(nc, mybir)
    return nc


def get_nc(bs=BS, c=C, h=H, w=W):
    key = (bs, c, h, w)
    if key not in _CACHE:
        _CACHE[key] = _build(bs, c, h, w)
    return _CACHE[key]


def kernel(x, wf, bf, wg, bg, wh, bh, **_unused):
    from concourse.bass_utils import run_bass_kernel_spmd

    x = np.ascontiguousarray(np.asarray(x, dtype=np.float32))
    wh = np.ascontiguousarray(np.asarray(wh, dtype=np.float32))
    bh = np.ascontiguousarray(np.asarray(bh, dtype=np.float32))

    in_maps = [
        {"x": x[k * BS : (k + 1) * BS], "wh": wh, "bh": bh} for k in range(N_CORES)
    ]
    # Tile scheduling is nondeterministic build-to-build and a rare schedule
    # can deadlock on hardware (NRT unrecoverable). Rebuilding produces a
    # fresh schedule, so retry with a clean build on any execution failure.
    last_err = None
    for attempt in range(3):
        try:
            nc = get_nc()
            res = run_bass_kernel_spmd(nc, in_maps, core_ids=list(range(N_CORES)))
            return np.concatenate(
                [res.results[k]["o"] for k in range(N_CORES)], axis=0
            )
        except Exception as e:  # rebuild with a new schedule and retry
            last_err = e
            _CACHE.clear()
    raise last_err
